# revision 3
# baseline (speedup 1.0000x reference)
"""HypergraphConv (HGCN) Trainium2 kernel.

Strategy (8 NeuronCores, zero collectives):
  - Host: sort the incidence list by destination (edge for phase 1, node for
    phase 2), shard the 400k entries across cores at destination boundaries
    (each core owns a disjoint edge/node range), and lay the per-entry source
    rows out as dense streams (pure data movement, no arithmetic).
  - Device kernel A: per-entry linear layer (x row * kron(W, I_T)) on the PE,
    then segment-sum into per-edge accumulators via one-hot matmuls into PSUM
    (uniform U_A tiles of 128 entries per 128-edge block), then scale by
    1/edge-degree. Outputs each core's own edge_feat rows.
  - Device kernel B: segment-sum of gathered edge rows (plus a HEWI column
    that accumulates into the node degree D) via one-hot matmuls, then scale
    by 1/D, add bias, ReLU. Outputs each core's own node rows.

Sharding: nnz dim across cores (as per the hint), but destination-sorted so
each core's partial sums are complete -> no all-reduce needed.
"""
import sys
import numpy as np

sys.path.insert(0, "/opt/trn_rl_repo")

import jax
from jax.sharding import Mesh, PartitionSpec
from jax.experimental.shard_map import shard_map

import concourse.bass as bass
import concourse.mybir as mybir
import concourse.tile as tile
from concourse.bass2jax import (
    _bass_exec_p,
    install_neuronx_cc_hook,
    partition_id_tensor,
)

# ---------------------------------------------------------------- tile patch
# This walrus build accepts only ONE sync-wait per instruction. Peel extra
# waits onto single-wait NOPs emitted just before, on the same engine.
import re as _re
from bass_rust import ScopedClock as _SC, VectorClock as _VC

_orig_add = tile.TileContext._add_instruction
_orig_drain = tile.TileContext._drain_and_barrier


def _split_add(self, inst):
    si = inst.sync_info
    if si is not None and si.on_wait and len(si.on_wait) > 1:
        waits = list(si.on_wait)
        if inst.engine != mybir.EngineType.Unassigned:
            for w in waits[:-1]:
                nop = mybir.InstNoOp(
                    name=self.nc.get_next_instruction_name(),
                    sync_info=mybir.SyncInfo(on_wait=[w], on_update=[]),
                    bass_nofuse=True,
                    engine=inst.engine,
                )
                _orig_add(self, nop)
            inst.sync_info = mybir.SyncInfo(
                on_wait=[waits[-1]], on_update=list(si.on_update or [])
            )
    _orig_add(self, inst)


def _patched_drain_and_barrier(self, tick_clock, wait_clock):
    gc = tick_clock.global_clock
    m = _re.search(r"\[([0-9, ]*)\]", repr(gc))
    vals = [int(x) for x in m.group(1).split(",") if x.strip() != ""]
    for idx, v in enumerate(vals):
        if v > 0:
            svc = _VC()
            svc.require_at_least(idx, v)
            nop = self.nc.sync.nop()
            wait_clock.add_sem_waits(nop.ins, _SC({None: svc}))
    self.nc.sync.drain()
    self.nc.all_engine_barrier()
    popped = self.nc._tile_sem_poison_stack.pop()
    assert popped is self._sem_poison
    self.nc.clear_and_free_semaphores(list(self.sems.allocated().values()))
    self.nc.all_engine_barrier()


tile.TileContext._add_instruction = _split_add
tile.TileContext._drain_and_barrier = _patched_drain_and_barrier

# ---------------------------------------------------------------- constants
NCORES = 8
B, N, F_IN, F_OUT, T = 4, 10000, 64, 4, 4
F_OUT = 64
NUM_NODES = B * N            # 40000
NUM_EDGES = 20000
NNZ = 400000
C = F_IN * T                 # 256 stream columns
FP = mybir.dt.float32
# PE matmul dtype: float32r streams fp32 at ~4x the fp32 rate on TRN2.
MM_DT = mybir.dt.float32

_RUNNERS = {}


# ---------------------------------------------------------------- programs
def _build_A(TA, UA, NBA, GA):
    nc = bass.Bass(target_bir_lowering=False)
    wk_in = nc.declare_dram_parameter("wk", [2, 128, C], FP, isOutput=False)
    iota_in = nc.declare_dram_parameter("iota", [128, 128], FP, isOutput=False)
    sA_in = nc.declare_dram_parameter("sA", [C, TA * 128], FP, isOutput=False)
    segA_in = nc.declare_dram_parameter("segA", [GA, 128, 16], FP, isOutput=False)
    out = nc.declare_dram_parameter("efA", [NBA * 128, C], FP, isOutput=True)

    with tile.TileContext(nc) as tc:
        with tc.tile_pool(name="const", bufs=1) as constp, \
             tc.tile_pool(name="lhs", bufs=6) as lhsp, \
             tc.tile_pool(name="xw", bufs=3) as xwp, \
             tc.tile_pool(name="oh", bufs=3) as ohp, \
             tc.tile_pool(name="seg", bufs=3) as segp, \
             tc.tile_pool(name="ost", bufs=2) as ostp, \
             tc.tile_pool(name="sm", bufs=4) as smp, \
             tc.tile_pool(name="pxw", bufs=2, space="PSUM") as pxwp, \
             tc.tile_pool(name="pseg", bufs=2, space="PSUM") as psegp:
            w0 = constp.tile([128, C], FP)
            w1 = constp.tile([128, C], FP)
            nc.sync.dma_start(out=w0[:], in_=wk_in[0])
            nc.sync.dma_start(out=w1[:], in_=wk_in[1])
            iota = constp.tile([128, 128], FP)
            nc.sync.dma_start(out=iota[:], in_=iota_in[:])

            segg = None
            pseg = None
            for t in range(TA):
                b, u = divmod(t, UA)
                g, j = divmod(t, 16)
                if j == 0:
                    segg = segp.tile([128, 16], FP)
                    nc.sync.dma_start(out=segg[:], in_=segA_in[g])
                l0 = lhsp.tile([128, 128], FP, tag="lhs")
                nc.sync.dma_start(out=l0[:], in_=sA_in[0:128, t * 128:(t + 1) * 128])
                l1 = lhsp.tile([128, 128], FP, tag="lhs")
                nc.sync.dma_start(out=l1[:], in_=sA_in[128:256, t * 128:(t + 1) * 128])
                pxw = pxwp.tile([128, C], FP)
                nc.tensor.matmul(out=pxw[:], lhsT=l0[:].bitcast(MM_DT), rhs=w0[:].bitcast(MM_DT), start=True, stop=False)
                nc.tensor.matmul(out=pxw[:], lhsT=l1[:].bitcast(MM_DT), rhs=w1[:].bitcast(MM_DT), start=False, stop=True)
                xw = xwp.tile([128, C + 1], FP)
                nc.vector.tensor_copy(out=xw[:, 0:C], in_=pxw[:])
                nc.vector.memset(xw[:, C:C + 1], 1.0)
                oh = ohp.tile([128, 128], FP)
                nc.vector.tensor_tensor(
                    out=oh[:],
                    in0=segg[:, j:j + 1].to_broadcast([128, 128]),
                    in1=iota[:],
                    op=mybir.AluOpType.is_equal,
                )
                if u == 0:
                    pseg = psegp.tile([128, C + 1], FP)
                nc.tensor.matmul(out=pseg[:], lhsT=oh[:].bitcast(MM_DT), rhs=xw[:].bitcast(MM_DT),
                                 start=(u == 0), stop=(u == UA - 1))
                if u == UA - 1:
                    cnt = smp.tile([128, 1], FP, tag="cnt")
                    nc.vector.tensor_scalar(
                        out=cnt[:], in0=pseg[:, C:C + 1],
                        scalar1=0.5, scalar2=None, op0=mybir.AluOpType.max,
                    )
                    inv = smp.tile([128, 1], FP, tag="inv")
                    nc.vector.reciprocal(out=inv[:], in_=cnt[:])
                    msk = smp.tile([128, 1], FP, tag="msk")
                    nc.vector.tensor_scalar(
                        out=msk[:], in0=pseg[:, C:C + 1],
                        scalar1=0.0, scalar2=None, op0=mybir.AluOpType.is_gt,
                    )
                    nc.vector.tensor_tensor(out=inv[:], in0=inv[:], in1=msk[:],
                                            op=mybir.AluOpType.mult)
                    ef = ostp.tile([128, C], FP)
                    nc.vector.tensor_tensor(
                        out=ef[:], in0=pseg[:, 0:C],
                        in1=inv[:, 0:1].to_broadcast([128, C]),
                        op=mybir.AluOpType.mult,
                    )
                    nc.sync.dma_start(out=out[b * 128:(b + 1) * 128, :], in_=ef[:])
    return nc


def _build_B(TB, UB, NBB, GB):
    CB = C + 1  # 256 features + HEWI column
    nc = bass.Bass(target_bir_lowering=False)
    iota_in = nc.declare_dram_parameter("iota", [128, 128], FP, isOutput=False)
    sB_in = nc.declare_dram_parameter("sB", [TB * 128, CB], FP, isOutput=False)
    segB_in = nc.declare_dram_parameter("segB", [GB, 128, 16], FP, isOutput=False)
    bias_in = nc.declare_dram_parameter("biasF", [128, C], FP, isOutput=False)
    out = nc.declare_dram_parameter("noB", [NBB * 128, C], FP, isOutput=True)

    with tile.TileContext(nc) as tc:
        with tc.tile_pool(name="const", bufs=1) as constp, \
             tc.tile_pool(name="rhs", bufs=6) as rhsp, \
             tc.tile_pool(name="oh", bufs=3) as ohp, \
             tc.tile_pool(name="seg", bufs=3) as segp, \
             tc.tile_pool(name="ost", bufs=2) as ostp, \
             tc.tile_pool(name="sm", bufs=4) as smp, \
             tc.tile_pool(name="pseg", bufs=2, space="PSUM") as psegp:
            iota = constp.tile([128, 128], FP)
            nc.sync.dma_start(out=iota[:], in_=iota_in[:])
            biasF = constp.tile([128, C], FP)
            nc.sync.dma_start(out=biasF[:], in_=bias_in[:])

            segg = None
            pseg = None
            for t in range(TB):
                b, u = divmod(t, UB)
                g, j = divmod(t, 16)
                if j == 0:
                    segg = segp.tile([128, 16], FP)
                    nc.sync.dma_start(out=segg[:], in_=segB_in[g])
                rhs = rhsp.tile([128, CB], FP)
                nc.sync.dma_start(out=rhs[:], in_=sB_in[t * 128:(t + 1) * 128, :])
                oh = ohp.tile([128, 128], FP)
                nc.vector.tensor_tensor(
                    out=oh[:],
                    in0=segg[:, j:j + 1].to_broadcast([128, 128]),
                    in1=iota[:],
                    op=mybir.AluOpType.is_equal,
                )
                if u == 0:
                    pseg = psegp.tile([128, CB], FP)
                nc.tensor.matmul(out=pseg[:], lhsT=oh[:].bitcast(MM_DT), rhs=rhs[:].bitcast(MM_DT),
                                 start=(u == 0), stop=(u == UB - 1))
                if u == UB - 1:
                    dsafe = smp.tile([128, 1], FP, tag="dsafe")
                    nc.vector.tensor_scalar(
                        out=dsafe[:], in0=pseg[:, C:C + 1],
                        scalar1=1e-30, scalar2=None, op0=mybir.AluOpType.max,
                    )
                    dinv = smp.tile([128, 1], FP, tag="dinv")
                    nc.vector.reciprocal(out=dinv[:], in_=dsafe[:])
                    msk = smp.tile([128, 1], FP, tag="msk")
                    nc.vector.tensor_scalar(
                        out=msk[:], in0=pseg[:, C:C + 1],
                        scalar1=0.0, scalar2=None, op0=mybir.AluOpType.is_gt,
                    )
                    nc.vector.tensor_tensor(out=dinv[:], in0=dinv[:], in1=msk[:],
                                            op=mybir.AluOpType.mult)
                    sc = ostp.tile([128, C], FP, tag="sc")
                    nc.vector.tensor_tensor(
                        out=sc[:], in0=pseg[:, 0:C],
                        in1=dinv[:, 0:1].to_broadcast([128, C]),
                        op=mybir.AluOpType.mult,
                    )
                    nc.vector.tensor_tensor(out=sc[:], in0=sc[:], in1=biasF[:],
                                            op=mybir.AluOpType.add)
                    nc.vector.tensor_scalar(
                        out=sc[:], in0=sc[:],
                        scalar1=0.0, scalar2=None, op0=mybir.AluOpType.max,
                    )
                    nc.sync.dma_start(out=out[b * 128:(b + 1) * 128, :], in_=sc[:])
    return nc


# ---------------------------------------------------------------- runner
class _Runner:
    def __init__(self, nc, n_cores=NCORES):
        install_neuronx_cc_hook()
        self.nc = nc
        self.n_cores = n_cores
        pname = nc.partition_id_tensor.name if nc.partition_id_tensor else None
        in_names, out_names, out_avals, zero_outs = [], [], [], []
        for alloc in nc.m.functions[0].allocations:
            if not isinstance(alloc, mybir.MemoryLocationSet):
                continue
            name = alloc.memorylocations[0].name
            if alloc.kind == "ExternalInput":
                if name != pname:
                    in_names.append(name)
            elif alloc.kind == "ExternalOutput":
                shape = tuple(alloc.tensor_shape)
                dtype = mybir.dt.np(alloc.dtype)
                out_names.append(name)
                out_avals.append(jax.core.ShapedArray(shape, dtype))
                zero_outs.append(np.zeros(shape, dtype))
        self.n_params = len(in_names)
        n_outs = len(out_avals)
        self.in_names = in_names + out_names
        if pname is not None:
            self.in_names.append(pname)
        self.out_names, self.out_avals, self.zero_outs = out_names, out_avals, zero_outs
        donate = tuple(range(self.n_params, self.n_params + n_outs))

        def _body(*args):
            operands = list(args)
            if pname is not None:
                operands.append(partition_id_tensor())
            return tuple(_bass_exec_p.bind(
                *operands,
                out_avals=tuple(out_avals),
                in_names=tuple(self.in_names),
                out_names=tuple(out_names),
                lowering_input_output_aliases=(),
                sim_require_finite=False,
                sim_require_nnan=False,
                nc=nc,
            ))

        devices = jax.devices()[:n_cores]
        mesh = Mesh(np.asarray(devices), ("core",))
        in_specs = (PartitionSpec("core"),) * (self.n_params + n_outs)
        out_specs = (PartitionSpec("core"),) * len(out_names)
        self.fn = jax.jit(
            shard_map(_body, mesh=mesh, in_specs=in_specs,
                      out_specs=out_specs, check_rep=False),
            donate_argnums=donate, keep_unused=True,
        )

    def run(self, in_maps):
        per_core = [
            [np.ascontiguousarray(m[name]) for name in self.in_names[: self.n_params]]
            for m in in_maps
        ]
        concat_in = [
            np.concatenate([per_core[c][i] for c in range(self.n_cores)], axis=0)
            for i in range(self.n_params)
        ]
        concat_zeros = [
            np.zeros((self.n_cores * z.shape[0], *z.shape[1:]), z.dtype)
            for z in self.zero_outs
        ]
        out_arrs = self.fn(*concat_in, *concat_zeros)
        jax.block_until_ready(out_arrs)
        return [
            {
                name: np.asarray(out_arrs[i]).reshape(
                    self.n_cores, *self.out_avals[i].shape
                )[c]
                for i, name in enumerate(self.out_names)
            }
            for c in range(self.n_cores)
        ]


# ---------------------------------------------------------------- host prep
def _plan(sorted_dst, n_dst_total):
    """Split destination-sorted entries into NCORES chunks at destination
    boundaries; compute uniform (NB, U) grid and per-core tile layouts."""
    nnz = len(sorted_dst)
    starts = []
    for c_ in range(NCORES):
        i = min(c_ * nnz // NCORES, nnz - 1)
        starts.append(int(np.searchsorted(sorted_dst, sorted_dst[i])))
    starts.append(nnz)
    # destination range per core
    dst_start = [int(sorted_dst[starts[c_]]) if starts[c_] < nnz else n_dst_total
                 for c_ in range(NCORES)]
    dst_start.append(n_dst_total)
    # block counts
    n_dst = [dst_start[c_ + 1] - dst_start[c_] for c_ in range(NCORES)]
    NB = max(1, max((nd + 127) // 128 for nd in n_dst))
    # entries per (core, block)
    U = 1
    per_core_blocks = []
    for c_ in range(NCORES):
        lo, hi = starts[c_], starts[c_ + 1]
        local = sorted_dst[lo:hi] - dst_start[c_]
        blk = local // 128
        counts = np.bincount(blk, minlength=NB)
        per_core_blocks.append((lo, hi, local, blk, counts))
        if counts.size:
            U = max(U, int((counts.max() + 127) // 128))
    return starts, dst_start, NB, U, per_core_blocks


def _layout(order, sorted_dst, per_core_blocks, dst_start, NB, U, src_rows, seg_dtype=np.float32):
    """Place entries on the uniform [NB*U, 128] grid.
    Returns per-core (row_gather_idx int64 with -1 for pad, seglocal [T,128])."""
    TT = NB * U
    outs = []
    for c_ in range(NCORES):
        lo, hi, local, blk, counts = per_core_blocks[c_]
        gidx = np.full(TT * 128, -1, np.int64)
        segl = np.full(TT * 128, -1.0, seg_dtype)
        # pack each block's entries into its U tiles
        off = np.zeros(NB + 1, np.int64)
        off[1:] = np.cumsum(counts)
        order_c = order[lo:hi]
        for b_ in range(NB):
            n_b = counts[b_] if b_ < len(counts) else 0
            if n_b == 0:
                continue
            base = b_ * U * 128
            sl = slice(off[b_], off[b_ + 1])
            gidx[base:base + n_b] = order_c[sl]
            segl[base:base + n_b] = (local[sl] - b_ * 128).astype(seg_dtype)
        outs.append((gidx, segl.reshape(TT, 128)))
    return outs


def _seg_groups(segl, TT):
    """[T,128] -> [G,128,16] with segl[t,p] at [t//16, p, t%16]."""
    G = (TT + 15) // 16
    segp = np.full((G * 16, 128), -1.0, np.float32)
    segp[:TT] = segl
    return np.ascontiguousarray(segp.reshape(G, 16, 128).transpose(0, 2, 1))


def kernel(x, HE, HEWI, W, b):
    x = np.asarray(x, np.float32)
    HE = np.asarray(HE)
    HEWI = np.asarray(HEWI, np.float32)
    W = np.asarray(W, np.float32)
    b = np.asarray(b, np.float32)

    xf = np.ascontiguousarray(x.reshape(NUM_NODES, C))        # (fi,t) cols
    wk = np.kron(W, np.eye(T, dtype=np.float32))              # [256,256]
    node_idx = HE[0].astype(np.int64)
    edge_idx = HE[1].astype(np.int64)
    iota = np.broadcast_to(np.arange(128, dtype=np.float32), (128, 128)).copy()

    # ---- phase A prep: sort by edge
    ordA = np.argsort(edge_idx, kind="stable")
    seA = edge_idx[ordA]
    startsA, e_start, NBA, UA, blocksA = _plan(seA, NUM_EDGES)
    TA = NBA * UA
    GA = (TA + 15) // 16
    layA = _layout(ordA, seA, blocksA, e_start, NBA, UA, xf)

    in_maps_A = []
    for c_ in range(NCORES):
        gidx, segl = layA[c_]
        rows = np.zeros((TA * 128, C), np.float32)
        valid = gidx >= 0
        rows[valid] = xf[node_idx[gidx[valid]]]
        sA = np.ascontiguousarray(rows.T)                     # [256, TA*128]
        in_maps_A.append({
            "wk": wk.reshape(2, 128, C),
            "iota": iota,
            "sA": sA,
            "segA": _seg_groups(segl, TA),
        })

    key_a = ("A", TA, UA, NBA, GA)
    if key_a not in _RUNNERS:
        _RUNNERS[key_a] = _Runner(_build_A(TA, UA, NBA, GA))
    resA = _RUNNERS[key_a].run(in_maps_A)

    edge_feat = np.zeros((NUM_EDGES, C), np.float32)
    for c_ in range(NCORES):
        ne = e_start[c_ + 1] - e_start[c_]
        if ne > 0:
            edge_feat[e_start[c_]:e_start[c_ + 1]] = resA[c_]["efA"][:ne]

    # ---- phase B prep: sort by node
    ordB = np.argsort(node_idx, kind="stable")
    snB = node_idx[ordB]
    startsB, v_start, NBB, UB, blocksB = _plan(snB, NUM_NODES)
    TB = NBB * UB
    GB = (TB + 15) // 16
    layB = _layout(ordB, snB, blocksB, v_start, NBB, UB, edge_feat)

    bexp = np.repeat(b, T).astype(np.float32)                 # [256] (fo-major)
    biasF = np.broadcast_to(bexp, (128, C)).copy()

    in_maps_B = []
    for c_ in range(NCORES):
        gidx, segl = layB[c_]
        rows = np.zeros((TB * 128, C + 1), np.float32)
        valid = gidx >= 0
        eidx = edge_idx[gidx[valid]]
        rows[valid, 0:C] = edge_feat[eidx]
        rows[valid, C] = HEWI[eidx]
        in_maps_B.append({
            "iota": iota,
            "sB": rows,
            "segB": _seg_groups(segl, TB),
            "biasF": biasF,
        })

    key_b = ("B", TB, UB, NBB, GB)
    if key_b not in _RUNNERS:
        _RUNNERS[key_b] = _Runner(_build_B(TB, UB, NBB, GB))
    resB = _RUNNERS[key_b].run(in_maps_B)

    node_out = np.zeros((NUM_NODES, C), np.float32)
    for c_ in range(NCORES):
        nv = v_start[c_ + 1] - v_start[c_]
        if nv > 0:
            node_out[v_start[c_]:v_start[c_ + 1]] = resB[c_]["noB"][:nv]

    return node_out.reshape(B, N, F_OUT, T)
